# revision 11
# baseline (speedup 1.0000x reference)
"""ChebyshevKANLayer on 8 Trainium2 NeuronCores.

y = silu(x) @ Wb + sum_d (x * T_d(xs)) @ Wc[:, :, d]
  xs = per-row rescale of x to [-1, 1]; T_d = Chebyshev polynomials.

Sharding: data-parallel over the batch dim (4096 -> 8 x 512 rows).
Weights are replicated. No collectives; host concatenates the shards.

Per-core kernel:
  - row stats (min/max) from the natural-layout shard, turned into
    per-row affine coefficients, broadcast into [128, 512] tiles via a
    PE transpose + K=1 ones-matmul,
  - Chebyshev recurrence evaluated directly on G_d = x*T_d in
    transposed layout ([in, batch]) on the vector engine
    (G_d = (2 xs) * G_{d-1} - G_{d-2}),
  - one fused contraction of depth 9*1024 over the 9 (activation,
    weight) pairs using float32r matmuls (full PE rate at N=512)
    accumulating into all 8 PSUM banks.
"""

import numpy as np

from concourse import bacc, masks, mybir, tile
from concourse.bass_utils import run_bass_kernel_spmd

B, IN, OUT, DEG = 4096, 1024, 1024, 8
NCORES = 8
BS = B // NCORES  # 512 rows per core
KT = IN // 128  # 8 contraction tiles
NB = BS // 128  # 4 batch tiles per core
NO = OUT // 512  # 2 output column tiles
NMAT = DEG + 1  # silu path + DEG chebyshev paths

F32 = mybir.dt.float32
F32R = mybir.dt.float32r
ALU = mybir.AluOpType
AF = mybir.ActivationFunctionType
AX = mybir.AxisListType


def _build_kernel(tc, out, xt, xn, wb, wc):
    nc = tc.nc
    from contextlib import ExitStack

    octx = ExitStack()
    const_pool = octx.enter_context(tc.tile_pool(name="const", bufs=1))
    ident = const_pool.tile([128, 128], F32)
    masks.make_identity(nc, ident[:])
    ones = const_pool.tile([1, 128], F32)
    nc.vector.memset(ones[:], 1.0)
    sb = const_pool.tile([128, BS], F32)  # broadcast of 2*s per column
    tb = const_pool.tile([128, BS], F32)  # broadcast of 2*t per column
    s_row = const_pool.tile([1, BS], F32)
    t_row = const_pool.tile([1, BS], F32)

    # --- stats phase: row min/max -> u = 2*xs = x*(4/(mx-mn)) + (-4*mn/(mx-mn) - 2)
    with (
        tc.tile_pool(name="stats", bufs=2) as spool,
        tc.tile_pool(name="psum_setup", bufs=1, space="PSUM") as pps,
    ):
        for t in range(NB):
            xnt = spool.tile([128, IN], F32, tag="xnt")
            nc.sync.dma_start(out=xnt[:], in_=xn[t * 128 : (t + 1) * 128, :])
            mx = spool.tile([128, 1], F32, tag="mx")
            mn = spool.tile([128, 1], F32, tag="mn")
            nc.vector.tensor_reduce(mx[:], xnt[:], axis=AX.X, op=ALU.max)
            nc.vector.tensor_reduce(mn[:], xnt[:], axis=AX.X, op=ALU.min)
            d = spool.tile([128, 1], F32, tag="d")
            nc.vector.tensor_tensor(d[:], mx[:], mn[:], ALU.subtract)
            r = spool.tile([128, 1], F32, tag="r")
            nc.vector.reciprocal(r[:], d[:])
            sc = spool.tile([128, 1], F32, tag="sc")
            # s2 = 4/(mx-mn)
            nc.vector.tensor_scalar(sc[:], r[:], 4.0, None, ALU.mult)
            # t2 = -mn*s2 - 2
            tmp = spool.tile([128, 1], F32, tag="tmp")
            nc.vector.tensor_tensor(tmp[:], mn[:], sc[:], ALU.mult)
            tcn = spool.tile([128, 1], F32, tag="tcn")
            nc.vector.tensor_scalar(tcn[:], tmp[:], -1.0, -2.0, ALU.mult, ALU.add)
            pst = pps.tile([1, 128], F32, tag="pst")
            nc.tensor.transpose(pst[:], sc[:], ident[:])
            nc.vector.tensor_copy(s_row[0:1, t * 128 : (t + 1) * 128], pst[:])
            pst2 = pps.tile([1, 128], F32, tag="pst2")
            nc.tensor.transpose(pst2[:], tcn[:], ident[:])
            nc.vector.tensor_copy(t_row[0:1, t * 128 : (t + 1) * 128], pst2[:])
        # broadcast the two stat rows across all 128 partitions
        psb = pps.tile([128, BS], F32, tag="psb")
        nc.tensor.matmul(psb[:], lhsT=ones[:], rhs=s_row[:], start=True, stop=True)
        nc.vector.tensor_copy(sb[:], psb[:])
        pstb = pps.tile([128, BS], F32, tag="pstb")
        nc.tensor.matmul(pstb[:], lhsT=ones[:], rhs=t_row[:], start=True, stop=True)
        nc.vector.tensor_copy(tb[:], pstb[:])

    # --- main phase ---
    with (
        tc.tile_pool(name="psum_acc", bufs=1, space="PSUM") as pacc,
        tc.tile_pool(name="w", bufs=2) as wpool,
        tc.tile_pool(name="g", bufs=2) as gpool,
        tc.tile_pool(name="xtp", bufs=2) as xtpool,
        tc.tile_pool(name="u", bufs=2) as upool,
        tc.tile_pool(name="o", bufs=2) as opool,
    ):
        po = [
            [
                pacc.tile([128, 512], F32, tag=f"po{t}{j}", name=f"po{t}{j}")
                for j in range(NO)
            ]
            for t in range(NB)
        ]
        for k in range(KT):
            ksl = slice(k * 128, (k + 1) * 128)
            xtt = xtpool.tile([128, BS], F32R, tag="xtt")
            nc.sync.dma_start(out=xtt[:], in_=xt[ksl, :])
            wall = wpool.tile([128, NMAT * OUT], F32R, tag="wall")
            nc.sync.dma_start(out=wall[:, 0:OUT], in_=wb[ksl, :])
            for dg in range(DEG):
                nc.sync.dma_start(
                    out=wall[:, (1 + dg) * OUT : (2 + dg) * OUT], in_=wc[dg, ksl, :]
                )
            gall = gpool.tile([128, DEG * BS], F32R, tag="gall")

            def Gs(i):
                return gall[:, i * BS : (i + 1) * BS]

            # slot 0: silu(x) = x*sigmoid(x); slots 1..7: G_1..G_7 (G_0 is xtt)
            nc.scalar.activation(Gs(0), xtt[:], AF.Sigmoid)
            nc.vector.tensor_tensor(Gs(0), Gs(0), xtt[:], ALU.mult)
            ut = upool.tile([128, BS], F32, tag="ut")
            nc.vector.tensor_tensor(ut[:], xtt[:], sb[:], ALU.mult)
            nc.vector.tensor_tensor(ut[:], ut[:], tb[:], ALU.add)
            # G_1 = x * xs = (x * 0.5) * u
            nc.vector.scalar_tensor_tensor(
                Gs(1), in0=xtt[:], scalar=0.5, in1=ut[:], op0=ALU.mult, op1=ALU.mult
            )
            for dg in range(2, DEG):
                tmpd = upool.tile([128, BS], F32, tag="tmpd")
                nc.vector.tensor_tensor(tmpd[:], ut[:], Gs(dg - 1), ALU.mult)
                prev2 = xtt[:] if dg == 2 else Gs(dg - 2)
                nc.vector.tensor_tensor(Gs(dg), tmpd[:], prev2, ALU.subtract)

            stats = [Gs(0), xtt[:]] + [Gs(i) for i in range(1, DEG)]
            for t in range(NB):
                for m in range(NMAT):
                    lhs = stats[m][:, t * 128 : (t + 1) * 128]
                    for j in range(NO):
                        rhs = wall[:, m * OUT + j * 512 : m * OUT + (j + 1) * 512]
                        nc.tensor.matmul(
                            po[t][j][:],
                            lhsT=lhs,
                            rhs=rhs,
                            start=(k == 0 and m == 0),
                            stop=(k == KT - 1 and m == NMAT - 1),
                        )
        for t in range(NB):
            for j in range(NO):
                ot = opool.tile([128, 512], F32, tag="ot")
                nc.vector.tensor_copy(ot[:], po[t][j][:])
                nc.sync.dma_start(
                    out=out[t * 128 : (t + 1) * 128, j * 512 : (j + 1) * 512],
                    in_=ot[:],
                )
    octx.close()


_NC_CACHE = None


def build_nc():
    global _NC_CACHE
    if _NC_CACHE is not None:
        return _NC_CACHE
    nc = bacc.Bacc(
        "TRN2", target_bir_lowering=False, debug=False, num_devices=NCORES
    )
    xt = nc.dram_tensor("xt", [IN, BS], F32R, kind="ExternalInput").ap()
    xn = nc.dram_tensor("xn", [BS, IN], F32, kind="ExternalInput").ap()
    wb = nc.dram_tensor("wb", [IN, OUT], F32R, kind="ExternalInput").ap()
    wc = nc.dram_tensor("wc", [DEG, IN, OUT], F32R, kind="ExternalInput").ap()
    out = nc.dram_tensor("out", [BS, OUT], F32, kind="ExternalOutput").ap()
    with tile.TileContext(nc) as tc:
        _build_kernel(tc, out, xt, xn, wb, wc)
    nc.compile()
    _NC_CACHE = nc
    return nc


def make_in_maps(x, base_weight, cheb_weight):
    x = np.ascontiguousarray(np.asarray(x, dtype=np.float32))
    wb = np.ascontiguousarray(np.asarray(base_weight, dtype=np.float32))
    wc = np.ascontiguousarray(
        np.asarray(cheb_weight, dtype=np.float32).transpose(2, 0, 1)
    )
    in_maps = []
    for c in range(NCORES):
        shard = x[c * BS : (c + 1) * BS]
        in_maps.append(
            {
                "xt": np.ascontiguousarray(shard.T),
                "xn": shard,
                "wb": wb,
                "wc": wc,
            }
        )
    return in_maps


def kernel(x, base_weight, cheb_weight, degree=DEG, **_):
    assert int(degree) == DEG
    nc = build_nc()
    in_maps = make_in_maps(x, base_weight, cheb_weight)
    res = run_bass_kernel_spmd(nc, in_maps, list(range(NCORES)))
    return np.concatenate([r["out"] for r in res.results], axis=0)


# revision 14
# speedup vs baseline: 100.9420x; 100.9420x over previous
"""ChebyshevKANLayer on 8 Trainium2 NeuronCores.

y = silu(x) @ Wb + sum_d (x * T_d(xs)) @ Wc[:, :, d]
  xs = per-row rescale of x to [-1, 1]; T_d = Chebyshev polynomials.

Sharding: data-parallel over the batch dim (4096 -> 8 x 512 rows).
Weights are replicated. No collectives; host concatenates the shards.

Per-core kernel:
  - row stats (min/max) from the natural-layout shard, turned into
    per-row affine coefficients, broadcast into [128, 512] tiles via a
    PE transpose + K=1 ones-matmul,
  - Chebyshev recurrence evaluated directly on G_d = x*T_d in
    transposed layout ([in, batch]) on the vector engine
    (G_d = (2 xs) * G_{d-1} - G_{d-2}),
  - one fused contraction of depth 9*1024 over the 9 (activation,
    weight) pairs using float32r matmuls (full PE rate at N=512)
    accumulating into all 8 PSUM banks.
"""

import numpy as np

from concourse import bacc, masks, mybir, tile
from concourse.bass_utils import run_bass_kernel_spmd

B, IN, OUT, DEG = 4096, 1024, 1024, 8
NCORES = 8
BS = B // NCORES  # 512 rows per core
KT = IN // 128  # 8 contraction tiles
NB = BS // 128  # 4 batch tiles per core
NO = OUT // 512  # 2 output column tiles
NMAT = DEG + 1  # silu path + DEG chebyshev paths

F32 = mybir.dt.float32
F32R = mybir.dt.float32r
ALU = mybir.AluOpType
AF = mybir.ActivationFunctionType
AX = mybir.AxisListType


def _build_kernel(tc, out, xt, xn, wb, wc, repeat=1):
    nc = tc.nc
    from contextlib import ExitStack

    octx = ExitStack()
    const_pool = octx.enter_context(tc.tile_pool(name="const", bufs=1))
    ident = const_pool.tile([128, 128], F32)
    masks.make_identity(nc, ident[:])
    ones = const_pool.tile([1, 128], F32)
    nc.vector.memset(ones[:], 1.0)
    sb = const_pool.tile([128, BS], F32)  # broadcast of 2*s per column
    tb = const_pool.tile([128, BS], F32)  # broadcast of 2*t per column
    s_row = const_pool.tile([1, BS], F32)
    t_row = const_pool.tile([1, BS], F32)

    # --- stats phase: row min/max -> u = 2*xs = x*(4/(mx-mn)) + (-4*mn/(mx-mn) - 2)
    with (
        tc.tile_pool(name="stats", bufs=2) as spool,
        tc.tile_pool(name="psum_setup", bufs=1, space="PSUM") as pps,
    ):
        for t in range(NB):
            xnt = spool.tile([128, IN], F32, tag="xnt")
            nc.sync.dma_start(out=xnt[:], in_=xn[t * 128 : (t + 1) * 128, :])
            mx = spool.tile([128, 1], F32, tag="mx")
            mn = spool.tile([128, 1], F32, tag="mn")
            nc.vector.tensor_reduce(mx[:], xnt[:], axis=AX.X, op=ALU.max)
            nc.vector.tensor_reduce(mn[:], xnt[:], axis=AX.X, op=ALU.min)
            d = spool.tile([128, 1], F32, tag="d")
            nc.vector.tensor_tensor(d[:], mx[:], mn[:], ALU.subtract)
            r = spool.tile([128, 1], F32, tag="r")
            nc.vector.reciprocal(r[:], d[:])
            sc = spool.tile([128, 1], F32, tag="sc")
            # s2 = 4/(mx-mn)
            nc.vector.tensor_scalar(sc[:], r[:], 4.0, None, ALU.mult)
            # t2 = -mn*s2 - 2
            tmp = spool.tile([128, 1], F32, tag="tmp")
            nc.vector.tensor_tensor(tmp[:], mn[:], sc[:], ALU.mult)
            tcn = spool.tile([128, 1], F32, tag="tcn")
            nc.vector.tensor_scalar(tcn[:], tmp[:], -1.0, -2.0, ALU.mult, ALU.add)
            pst = pps.tile([1, 128], F32, tag="pst")
            nc.tensor.transpose(pst[:], sc[:], ident[:])
            nc.vector.tensor_copy(s_row[0:1, t * 128 : (t + 1) * 128], pst[:])
            pst2 = pps.tile([1, 128], F32, tag="pst2")
            nc.tensor.transpose(pst2[:], tcn[:], ident[:])
            nc.vector.tensor_copy(t_row[0:1, t * 128 : (t + 1) * 128], pst2[:])
        # broadcast the two stat rows across all 128 partitions
        psb = pps.tile([128, BS], F32, tag="psb")
        nc.tensor.matmul(psb[:], lhsT=ones[:], rhs=s_row[:], start=True, stop=True)
        nc.vector.tensor_copy(sb[:], psb[:])
        pstb = pps.tile([128, BS], F32, tag="pstb")
        nc.tensor.matmul(pstb[:], lhsT=ones[:], rhs=t_row[:], start=True, stop=True)
        nc.vector.tensor_copy(tb[:], pstb[:])

    # --- main phase ---
    with (
        tc.tile_pool(name="psum_acc", bufs=1, space="PSUM") as pacc,
        tc.tile_pool(name="w", bufs=2) as wpool,
        tc.tile_pool(name="g", bufs=2) as gpool,
        tc.tile_pool(name="xtp", bufs=2) as xtpool,
        tc.tile_pool(name="u", bufs=2) as upool,
        tc.tile_pool(name="o", bufs=2) as opool,
    ):
        po = [
            [
                pacc.tile([128, 512], F32, tag=f"po{t}{j}", name=f"po{t}{j}")
                for j in range(NO)
            ]
            for t in range(NB)
        ]
        for rep in range(repeat):
          for k in range(KT):
            ksl = slice(k * 128, (k + 1) * 128)
            xtt = xtpool.tile([128, BS], F32R, tag="xtt")
            nc.sync.dma_start(out=xtt[:], in_=xt[ksl, :])
            wall = wpool.tile([128, NMAT * OUT], F32R, tag="wall")
            nc.sync.dma_start(out=wall[:, 0:OUT], in_=wb[ksl, :])
            for dg in range(DEG):
                nc.sync.dma_start(
                    out=wall[:, (1 + dg) * OUT : (2 + dg) * OUT], in_=wc[dg, ksl, :]
                )
            gall = gpool.tile([128, DEG * BS], F32R, tag="gall")

            def Gs(i):
                return gall[:, i * BS : (i + 1) * BS]

            # slot 0: silu(x) = x*sigmoid(x); slots 1..7: G_1..G_7 (G_0 is xtt)
            nc.scalar.activation(Gs(0), xtt[:], AF.Sigmoid)
            nc.vector.tensor_tensor(Gs(0), Gs(0), xtt[:], ALU.mult)
            ut = upool.tile([128, BS], F32, tag="ut")
            nc.vector.tensor_tensor(ut[:], xtt[:], sb[:], ALU.mult)
            nc.vector.tensor_tensor(ut[:], ut[:], tb[:], ALU.add)
            # G_1 = x * xs = (x * 0.5) * u
            nc.vector.scalar_tensor_tensor(
                Gs(1), in0=xtt[:], scalar=0.5, in1=ut[:], op0=ALU.mult, op1=ALU.mult
            )
            for dg in range(2, DEG):
                tmpd = upool.tile([128, BS], F32, tag="tmpd")
                nc.vector.tensor_tensor(tmpd[:], ut[:], Gs(dg - 1), ALU.mult)
                prev2 = xtt[:] if dg == 2 else Gs(dg - 2)
                nc.vector.tensor_tensor(Gs(dg), tmpd[:], prev2, ALU.subtract)

            stats = [Gs(0), xtt[:]] + [Gs(i) for i in range(1, DEG)]
            for t in range(NB):
                for m in range(NMAT):
                    lhs = stats[m][:, t * 128 : (t + 1) * 128]
                    for j in range(NO):
                        rhs = wall[:, m * OUT + j * 512 : m * OUT + (j + 1) * 512]
                        nc.tensor.matmul(
                            po[t][j][:],
                            lhsT=lhs,
                            rhs=rhs,
                            start=(k == 0 and m == 0),
                            stop=(k == KT - 1 and m == NMAT - 1),
                        )
        for t in range(NB):
            for j in range(NO):
                ot = opool.tile([128, 512], F32, tag="ot")
                nc.vector.tensor_copy(ot[:], po[t][j][:])
                nc.sync.dma_start(
                    out=out[t * 128 : (t + 1) * 128, j * 512 : (j + 1) * 512],
                    in_=ot[:],
                )
    octx.close()


_NC_CACHE = {}


def build_nc(repeat=1):
    if repeat in _NC_CACHE:
        return _NC_CACHE[repeat]
    nc = bacc.Bacc(
        "TRN2", target_bir_lowering=False, debug=False, num_devices=NCORES
    )
    xt = nc.dram_tensor("xt", [IN, BS], F32R, kind="ExternalInput").ap()
    xn = nc.dram_tensor("xn", [BS, IN], F32, kind="ExternalInput").ap()
    wb = nc.dram_tensor("wb", [IN, OUT], F32R, kind="ExternalInput").ap()
    wc = nc.dram_tensor("wc", [DEG, IN, OUT], F32R, kind="ExternalInput").ap()
    out = nc.dram_tensor("out", [BS, OUT], F32, kind="ExternalOutput").ap()
    with tile.TileContext(nc) as tc:
        _build_kernel(tc, out, xt, xn, wb, wc, repeat=repeat)
    nc.compile()
    _NC_CACHE[repeat] = nc
    return nc


def make_in_maps(x, base_weight, cheb_weight):
    x = np.ascontiguousarray(np.asarray(x, dtype=np.float32))
    wb = np.ascontiguousarray(np.asarray(base_weight, dtype=np.float32))
    wc = np.ascontiguousarray(
        np.asarray(cheb_weight, dtype=np.float32).transpose(2, 0, 1)
    )
    in_maps = []
    for c in range(NCORES):
        shard = x[c * BS : (c + 1) * BS]
        in_maps.append(
            {
                "xt": np.ascontiguousarray(shard.T),
                "xn": shard,
                "wb": wb,
                "wc": wc,
            }
        )
    return in_maps


def kernel(x, base_weight, cheb_weight, degree=DEG, **_):
    assert int(degree) == DEG
    nc = build_nc()
    in_maps = make_in_maps(x, base_weight, cheb_weight)
    res = run_bass_kernel_spmd(nc, in_maps, list(range(NCORES)))
    return np.concatenate([r["out"] for r in res.results], axis=0)
